# revision 12
# baseline (speedup 1.0000x reference)
"""Causal multi-head attention (B=2, S=2048, E=1024, H=16) on 8 TRN2 NeuronCores.

Sharding: 8 cores = 2 batches x 4 head-groups (4 heads / 256 dims each).
Each core loads its batch's q/k/v (pre-transposed to [E, S] on host), its
head-group's Wq/Wk/Wv column-slices and Wo row-slice, computes projections +
causal attention + a partial output projection [S, E] in fp16; the host sums
the 4 partials per batch and adds the bias.

v3: fp8 DoubleRow matmuls where the error budget allows, with hi/lo
splitting to cancel quantization error on the stationary operand:
- Q/K projections: stationary = (fp8_hi, fp8_lo) of 8*W (exact to fp8
  residual), moving = fp8(x) streamed into both slots via a stride-0 AP.
  2x PE throughput; only x's fp8 rounding (~1e-2 rel) survives.
- attn@V: stationary = (V_hi, V_lo) fp8 pair + a ones/zeros column that
  rides the denominator; moving = fp8 exp-probs (et) streamed twice.
- Scores stay fp16 (two heads packed into disjoint 64-row PE groups);
  V/O projections stay fp16 (O-path errors do not attenuate).
Softmax is max-free in the transposed [t, s] domain; the causal mask is a
-30000 pattern accumulated by an identity matmul (exp -> exact 0 in fp8).
Score PSUM is one [128, 4, 512] tile (2 t-tiles x 2 heads) so each exp is a
single 2048-column activation (ACT is the roofline-critical engine).
"""

import numpy as np

B, S, E, H, D = 2, 2048, 1024, 16, 64
HPC = 4              # heads per core
L = HPC * D          # 256 local dims per core
SB = 512             # s-block (softmax/matmul free-dim block)
NSB = S // SB        # 4
TT = 128             # t-tile
NTT = S // TT        # 16
NET = E // 128       # 8 contraction tiles over E

_cache = {}


def _patch_tile_drain():
    """This container's walrus allows only one sync-wait per instruction.
    Split the TileContext tail-drain waits across standalone SP nops."""
    import bass_rust
    import concourse.tile as tile
    from concourse.vector_clock import ScopedClock

    if getattr(tile.TileContext, "_drain_patched", False):
        return

    def _drain_and_barrier(self, tick_clock, wait_clock):
        drain_inst = self.nc.sync.drain()
        wait_clock.add_sem_waits(
            drain_inst.ins, ScopedClock({None: tick_clock.global_clock})
        )
        si = drain_inst.ins.sync_info
        if si is not None and len(si.on_wait) > 1:
            waits = list(si.on_wait)
            drain_inst.ins.sync_info = bass_rust.SyncInfo(
                on_wait=[waits[0]], on_update=list(si.on_update)
            )
            for w in waits[1:]:
                nop = self.nc.sync.nop(nofuse=True)
                nop.ins.sync_info = bass_rust.SyncInfo(on_wait=[w], on_update=[])
        self.nc.all_engine_barrier()
        assert self.sems is not None
        popped = self.nc._tile_sem_poison_stack.pop()
        assert popped is self._sem_poison
        self.nc.clear_and_free_semaphores(list(self.sems.allocated().values()))
        self.nc.all_engine_barrier()

    tile.TileContext._drain_and_barrier = _drain_and_barrier
    tile.TileContext._drain_patched = True


def _split_multi_waits(nc):
    """Move extra per-instruction semaphore waits onto standalone same-engine
    NoOps inserted immediately before the instruction (walrus 1-wait limit)."""
    import bass_rust

    def make_wait_nop(engine_ty, wait):
        eng = None
        for e in (nc.tensor, nc.scalar, nc.vector, nc.gpsimd, nc.sync):
            if e.engine == engine_ty:
                eng = e
                break
        assert eng is not None, f"no engine object for {engine_ty}"
        bi = eng.nop(nofuse=True)
        inst = bi.ins
        bb = nc.cur_bb.bb if nc.cur_bb is not None else None
        if bb is not None and bb.instructions and bb.instructions[-1] is inst:
            bb.instructions.pop()
        inst.sync_info = bass_rust.SyncInfo(on_wait=[wait], on_update=[])
        return inst

    f = nc.m.functions[0]
    for blk in f.blocks:
        new_list = []
        changed = False
        for inst in blk.instructions:
            si = inst.sync_info
            if si is not None and len(si.on_wait) > 1:
                waits = list(si.on_wait)
                for w in waits[:-1]:
                    new_list.append(make_wait_nop(inst.engine, w))
                inst.sync_info = bass_rust.SyncInfo(
                    on_wait=[waits[-1]], on_update=list(si.on_update)
                )
                changed = True
            new_list.append(inst)
        if changed:
            blk.instructions = new_list


def _build(repeat=1):
    import concourse.bass as bass
    import concourse.tile as tile
    from concourse import mybir

    _patch_tile_drain()

    f32 = mybir.dt.float32
    f16 = mybir.dt.float16
    f8 = mybir.dt.float8e4
    EXP = mybir.ActivationFunctionType.Exp
    MULT = mybir.AluOpType.mult
    SUB = mybir.AluOpType.subtract
    DR = mybir.MatmulPerfMode.DoubleRow
    ESCALE = 0.125 / 64.0  # 1/sqrt(D) with the 8x weight scale folded out

    nc = bass.Bass()
    q8 = nc.declare_dram_parameter("q8", [E, S], f8, isOutput=False)
    k8 = nc.declare_dram_parameter("k8", [E, S], f8, isOutput=False)
    vT = nc.declare_dram_parameter("vT", [E, S], f16, isOutput=False)
    wqhl = nc.declare_dram_parameter("wqhl", [E, 2, L], f8, isOutput=False)
    wkhl = nc.declare_dram_parameter("wkhl", [E, 2, L], f8, isOutput=False)
    wv = nc.declare_dram_parameter("wv", [E, L], f16, isOutput=False)
    wo = nc.declare_dram_parameter("wo", [L, E], f16, isOutput=False)
    ident = nc.declare_dram_parameter("ident", [128, 128], f16, isOutput=False)
    bn128 = nc.declare_dram_parameter("bn128", [128, 128], f16, isOutput=False)
    out = nc.declare_dram_parameter("out", [S, E], f16, isOutput=True)

    with tile.TileContext(nc) as tc:
        with (
            tc.tile_pool(name="const", bufs=1) as const,
            tc.tile_pool(name="resid", bufs=1) as resid,
            tc.tile_pool(name="x_in", bufs=1) as x_in,
            tc.tile_pool(name="v_in", bufs=3) as v_in,
            tc.tile_pool(name="expp", bufs=4) as expp,
            tc.tile_pool(name="sm", bufs=3) as smp,
            tc.tile_pool(name="outp", bufs=3) as outp,
        ):
            # ---- resident weights / masks ----
            wq_sb = const.tile([128, NET, 2, L], f8, tag="wq")
            wk_sb = const.tile([128, NET, 2, L], f8, tag="wk")
            wv_sb = const.tile([128, NET, L], f16, tag="wv")
            wo_sb = const.tile([128, 2, E], f16, tag="wo")
            id_sb = const.tile([128, 128], f16, tag="ident")
            bn_sb = const.tile([128, 128], f16, tag="bn128")
            ones_f32 = const.tile([128, 1], f32, tag="ones32")
            ones_r = const.tile([1, 64], f16, tag="onesr")
            nc.gpsimd.dma_start(
                wq_sb[:], wqhl.rearrange("(n p) two l -> p n two l", p=128)
            )
            nc.gpsimd.dma_start(
                wk_sb[:], wkhl.rearrange("(n p) two l -> p n two l", p=128)
            )
            nc.gpsimd.dma_start(wv_sb[:], wv.rearrange("(n p) l -> p n l", p=128))
            nc.gpsimd.dma_start(id_sb[:], ident[:])
            nc.gpsimd.dma_start(bn_sb[:], bn128[:])
            nc.gpsimd.dma_start(wo_sb[:], wo.rearrange("(h p) e -> p h e", p=128))
            nc.any.memset(ones_f32[:], 1.0)
            nc.vector.tensor_copy(
                ones_r[:], ones_f32[0:1, :].broadcast_to([1, 64])
            )

            # ---- residents ----
            QT = resid.tile([128, 2, S], f16, tag="QT")   # [dim%128, hp, s], 8x
            KT = resid.tile([128, 2, S], f16, tag="KT")
            # V hi/lo fp8 + denominator column: slot 0 = hi (+ones col),
            # slot 1 = lo residual (+zeros col); cols 65:68 pad for 16B steps
            Vn = resid.tile([128, NTT, 2, HPC, 68], f8, tag="Vn")
            PT = resid.tile([128, 2, S], f16, tag="PT")   # normalized attn out.T
            nc.any.memset(Vn[:], 0.0)
            nc.vector.tensor_copy(
                Vn[:, :, 0, :, 64:65],
                ones_f32[:, None, None, :].broadcast_to([128, NTT, HPC, 1]),
            )

            for _rep in range(repeat):
                with (
                    tc.tile_pool(name="ps_sc", bufs=1, space="PSUM") as ps_sc,
                    tc.tile_pool(name="ps_av", bufs=1, space="PSUM") as ps_av,
                    tc.tile_pool(name="ps_gen", bufs=2, space="PSUM") as ps_gen,
                ):
                    xq8 = x_in.tile([128, NET, S], f8, tag="xq")
                    xk8 = x_in.tile([128, NET, S], f8, tag="xk")

                    def load_x(x_sb, x_dram):
                        for half in range(2):
                            ks = slice(half * 4, (half + 1) * 4)
                            nc.sync.dma_start(
                                x_sb[:, ks, :],
                                x_dram.rearrange("(n p) s -> p n s", p=128)[:, ks, :],
                            )

                    def qk_proj_pair(sb_pair):
                        # stationary (w hi/lo) reused across the sb pair so
                        # the 256-col DoubleRow LDWEIGHTS hides under 2 MMs
                        for x8, w_sb, dst in (
                            (xq8, wq_sb, QT),
                            (xk8, wk_sb, KT),
                        ):
                            for hp in range(2):
                                pss = [
                                    (
                                        sb,
                                        ps_gen.tile(
                                            [128, SB], f32, tag="pp", name="pp"
                                        ),
                                    )
                                    for sb in sb_pair
                                ]
                                for kt in range(NET):
                                    for sb, ps in pss:
                                        nc.tensor.matmul(
                                            ps[:],
                                            w_sb[:, kt, :, hp * 128 : (hp + 1) * 128],
                                            x8[
                                                :, kt, None, sb * SB : (sb + 1) * SB
                                            ].broadcast_to([128, 2, SB]),
                                            start=(kt == 0),
                                            stop=(kt == NET - 1),
                                            perf_mode=DR,
                                        )
                                for sb, ps in pss:
                                    nc.vector.tensor_copy(
                                        dst[:, hp, sb * SB : (sb + 1) * SB], ps[:]
                                    )

                    def v_proj(tt):
                        vt = v_in.tile([128, NET, TT], f16, tag="vt")
                        nc.sync.dma_start(
                            vt[:],
                            vT.rearrange("(n p) s -> p n s", p=128)[
                                :, :, tt * TT : (tt + 1) * TT
                            ],
                        )
                        psf = ps_gen.tile([128, SB], f32, tag="pp")
                        ps = psf[:, 0:L]
                        for kk in range(NET):
                            nc.tensor.matmul(
                                ps[:],
                                vt[:, kk, :],
                                wv_sb[:, kk, :],
                                start=(kk == 0),
                                stop=(kk == NET - 1),
                            )
                        src = ps[:].rearrange("p (h d) -> p h d", d=64)
                        nc.vector.tensor_copy(Vn[:, tt, 0, :, 0:64], src)
                        nc.vector.tensor_tensor(
                            Vn[:, tt, 1, :, 0:64], src, Vn[:, tt, 0, :, 0:64], op=SUB
                        )

                    def out_proj(st_list):
                        for st in st_list:
                            for eb in range(E // SB):
                                pso = ps_gen.tile([128, SB], f32, tag="pp")
                                for hp in range(2):
                                    nc.tensor.matmul(
                                        pso[:],
                                        PT[:, hp, st * 128 : (st + 1) * 128],
                                        wo_sb[:, hp, eb * SB : (eb + 1) * SB],
                                        start=(hp == 0),
                                        stop=(hp == 1),
                                    )
                                ot = outp.tile([128, SB], f16, tag="ot")
                                nc.vector.tensor_copy(ot[:], pso[:])
                                nc.sync.dma_start(
                                    out[
                                        st * 128 : (st + 1) * 128,
                                        eb * SB : (eb + 1) * SB,
                                    ],
                                    ot[:],
                                )

                    load_x(xq8, q8)
                    load_x(xk8, k8)
                    for sb in range(NSB):
                        sbs = slice(sb * SB, (sb + 1) * SB)
                        if sb == 0:
                            qk_proj_pair((0, 1))
                        for tt in range(4 * sb, 4 * (sb + 1)):
                            v_proj(tt)
                        if sb == 1:
                            qk_proj_pair((2, 3))

                        # ---- attention for this s-block ----
                        n_tt = (sb + 1) * (SB // TT)  # causal t-tiles
                        for hp in range(2):
                            av0 = ps_av.tile([128, SB], f32, tag="av0")
                            av1 = ps_av.tile([128, SB], f32, tag="av1")
                            for g in range(n_tt // 2):  # pairs of t-tiles
                                # [t, 2 t-tiles x 2 heads, s] score tile:
                                # slots 0,1 = head-even j=0,1; 2,3 = head-odd
                                sc = ps_sc.tile([128, 4, SB], f32, tag="sc")
                                for j in range(2):
                                    tt = 2 * g + j
                                    tts = slice(tt * TT, (tt + 1) * TT)
                                    if tt >= sb * 4:
                                        # diagonal: scores start the group on
                                        # the valid cols; a [128,128] -30000
                                        # above-diag pattern accumulates onto
                                        # the triangle block. Cols [0:no) are
                                        # left unwritten -> exp garbage there
                                        # is never read (AV streams [no:SB)).
                                        kk = tt - sb * 4
                                        no = kk * TT
                                        for h01 in range(2):
                                            nc.tensor.matmul(
                                                sc[:, 2 * h01 + j, no:SB],
                                                KT[
                                                    64 * h01 : 64 * h01 + 64,
                                                    hp, tts,
                                                ],
                                                QT[
                                                    64 * h01 : 64 * h01 + 64,
                                                    hp,
                                                    sb * SB + no : (sb + 1) * SB,
                                                ],
                                                start=True, stop=False,
                                            )
                                            nc.tensor.matmul(
                                                sc[:, 2 * h01 + j, no : no + TT],
                                                id_sb[:], bn_sb[:],
                                                start=False, stop=True,
                                            )
                                    else:
                                        # heads pack into disjoint PE rows
                                        for h01 in range(2):
                                            nc.tensor.matmul(
                                                sc[:, 2 * h01 + j, :],
                                                KT[
                                                    64 * h01 : 64 * h01 + 64,
                                                    hp, tts,
                                                ],
                                                QT[64 * h01 : 64 * h01 + 64, hp, sbs],
                                                start=True, stop=True,
                                            )
                                et = expp.tile([128, 4, SB], f8, tag="et")
                                # restrict exp to the pair's valid col union
                                npr = max(0, (2 * g - sb * 4) * TT)
                                nc.scalar.activation(
                                    et[:, :, npr:SB], sc[:, :, npr:SB],
                                    EXP, scale=ESCALE,
                                )
                                for h01 in range(2):
                                    av = av0 if h01 == 0 else av1
                                    for j in range(2):
                                        tt = 2 * g + j
                                        st_f = g == 0 and j == 0
                                        sp_f = g == n_tt // 2 - 1 and j == 1
                                        no = max(0, (tt - sb * 4) * TT)
                                        nc.tensor.matmul(
                                            av[0:65, no:SB],
                                            Vn[:, tt, :, 2 * hp + h01, 0:65],
                                            et[
                                                :, 2 * h01 + j, None, no:SB
                                            ].broadcast_to([128, 2, SB - no]),
                                            start=st_f, stop=sp_f,
                                            perf_mode=DR,
                                        )
                            # normalize: PT[po:po+64, hp, sbs] = av[0:64]/av[64]
                            for po, av in ((0, av0), (64, av1)):
                                rdh = smp.tile([1, SB], f16, tag="rdh")
                                with nc.allow_low_precision(
                                    reason="1/den fits fp16; error averages out"
                                ):
                                    nc.vector.reciprocal(rdh[:], av[64:65, :])
                                # broadcast 1/den across 64 partitions via a
                                # K=1 fp16 matmul into the upper half
                                nc.tensor.matmul(
                                    av[64:128, :], ones_r[:], rdh[:],
                                    start=True, stop=True,
                                )
                                bcs = smp.tile([64, SB], f32, tag="bcs")
                                nc.vector.tensor_copy(bcs[:], av[64:128, :])
                                nc.vector.tensor_tensor(
                                    PT[po : po + 64, hp, sbs],
                                    av[0:64, :], bcs[:], op=MULT,
                                )

                        # ---- output projection (one s-block delayed so
                        # it stays available as PE filler; last block inline)
                        if sb > 0:
                            out_proj(
                                list(range(4 * (sb - 1), 4 * sb))
                                + (list(range(12, 16)) if sb == 3 else [])
                            )

    _split_multi_waits(nc)
    return nc


def _get_nc():
    if "nc" not in _cache:
        _cache["nc"] = _build()
    return _cache["nc"]


def _make_runner(nc, n_cores=8):
    """Build a cached jitted SPMD executor (jit once; warm calls are cheap)."""
    import jax
    from jax.sharding import Mesh, PartitionSpec
    from jax.experimental.shard_map import shard_map

    from concourse import mybir
    from concourse.bass2jax import (
        _bass_exec_p,
        install_neuronx_cc_hook,
        partition_id_tensor,
    )

    install_neuronx_cc_hook()
    partition_name = nc.partition_id_tensor.name if nc.partition_id_tensor else None
    in_names, out_names, out_avals, zero_outs = [], [], [], []
    for alloc in nc.m.functions[0].allocations:
        if not isinstance(alloc, mybir.MemoryLocationSet):
            continue
        name = alloc.memorylocations[0].name
        if alloc.kind == "ExternalInput":
            if name != partition_name:
                in_names.append(name)
        elif alloc.kind == "ExternalOutput":
            shape = tuple(alloc.tensor_shape)
            dtype = mybir.dt.np(alloc.dtype)
            out_names.append(name)
            out_avals.append(jax.core.ShapedArray(shape, dtype))
            zero_outs.append(np.zeros(shape, dtype))
    n_params = len(in_names)
    all_in_names = list(in_names) + list(out_names)
    if partition_name is not None:
        all_in_names.append(partition_name)

    def _body(*args):
        operands = list(args)
        if partition_name is not None:
            operands.append(partition_id_tensor())
        return tuple(
            _bass_exec_p.bind(
                *operands,
                out_avals=tuple(out_avals),
                in_names=tuple(all_in_names),
                out_names=tuple(out_names),
                lowering_input_output_aliases=(),
                sim_require_finite=True,
                sim_require_nnan=True,
                nc=nc,
            )
        )

    devices = jax.devices()[:n_cores]
    mesh = Mesh(np.asarray(devices), ("core",))
    in_specs = (PartitionSpec("core"),) * (n_params + len(out_names))
    out_specs = (PartitionSpec("core"),) * len(out_names)
    fn = jax.jit(
        shard_map(
            _body, mesh=mesh, in_specs=in_specs, out_specs=out_specs, check_rep=False
        ),
        keep_unused=True,
    )

    def run(in_maps):
        arrs = [
            np.concatenate([np.asarray(m[name]) for m in in_maps], axis=0)
            for name in in_names
        ]
        zeros = [
            np.zeros((n_cores * z.shape[0], *z.shape[1:]), z.dtype)
            for z in zero_outs
        ]
        outs = fn(*arrs, *zeros)
        per_core = []
        for c in range(n_cores):
            d = {}
            for i, name in enumerate(out_names):
                full = np.asarray(outs[i])
                d[name] = full.reshape(n_cores, full.shape[0] // n_cores, *full.shape[1:])[c]
            per_core.append(d)
        return per_core

    return run


def _get_runner():
    if "run" not in _cache:
        _cache["run"] = _make_runner(_get_nc())
    return _cache["run"]


def _host_inputs(q, k, v, Wq, Wk, Wv, Wo):
    import ml_dtypes

    F8 = ml_dtypes.float8_e4m3
    q = np.asarray(q, dtype=np.float32)
    k = np.asarray(k, dtype=np.float32)
    v = np.asarray(v, dtype=np.float32)
    WoT = np.asarray(Wo, dtype=np.float32).T

    q8b = [q[b].T.astype(F8) for b in range(B)]
    k8b = [k[b].T.astype(F8) for b in range(B)]
    vTb = [v[b].T.astype(np.float16) for b in range(B)]

    def hilo(w):  # [E, L] fp32 -> [E, 2, L] fp8 (hi, residual)
        hi = w.astype(F8)
        lo = (w - hi.astype(np.float32)).astype(F8)
        return np.stack([hi, lo], axis=1)

    wqhl = [hilo(8.0 * np.asarray(Wq, np.float32)[g * L : (g + 1) * L, :].T)
            for g in range(4)]
    wkhl = [hilo(8.0 * np.asarray(Wk, np.float32)[g * L : (g + 1) * L, :].T)
            for g in range(4)]
    wvT = [np.asarray(Wv, np.float32)[g * L : (g + 1) * L, :].T.astype(np.float16)
           for g in range(4)]
    woT = [WoT[g * L : (g + 1) * L, :].astype(np.float16) for g in range(4)]

    ti = np.arange(128)[:, None]
    sj = np.arange(128)[None, :]
    ident = np.eye(128, dtype=np.float16)
    bn128 = np.where(ti > sj, np.float16(-30000.0), np.float16(0.0))

    in_maps = []
    for c in range(8):
        b, g = c // 4, c % 4
        in_maps.append(
            {
                "q8": q8b[b], "k8": k8b[b], "vT": vTb[b],
                "wqhl": wqhl[g], "wkhl": wkhl[g], "wv": wvT[g],
                "wo": woT[g], "ident": ident, "bn128": bn128,
            }
        )
    return in_maps


def kernel(q, k, v, Wq, Wk, Wv, Wo, bo):
    run = _get_runner()
    in_maps = _host_inputs(q, k, v, Wq, Wk, Wv, Wo)
    res = run(in_maps)
    out = np.empty((B, S, E), dtype=np.float32)
    bo = np.asarray(bo, dtype=np.float32)
    for b in range(B):
        acc = res[4 * b]["out"].astype(np.float32)
        for g in range(1, 4):
            acc = acc + res[4 * b + g]["out"].astype(np.float32)
        out[b] = acc + bo[None, :]
    return out


# revision 19
# speedup vs baseline: 1.6147x; 1.6147x over previous
"""Causal multi-head attention (B=2, S=2048, E=1024, H=16) on 8 TRN2 NeuronCores.

Sharding: 8 cores = 2 batches x 4 head-groups (4 heads / 256 dims each).
Each core loads its batch's q/k/v (pre-transposed to [E, S] on host), its
head-group's Wq/Wk/Wv column-slices and Wo row-slice, computes projections +
causal attention + a partial output projection [S, E] in fp16; the host sums
the 4 partials per batch and adds the bias.

v3: fp8 DoubleRow matmuls where the error budget allows, with hi/lo
splitting to cancel quantization error on the stationary operand:
- Q/K projections: stationary = (fp8_hi, fp8_lo) of 8*W (exact to fp8
  residual), moving = fp8(x) streamed into both slots via a stride-0 AP.
  2x PE throughput; only x's fp8 rounding (~1e-2 rel) survives.
- attn@V: stationary = (V_hi, V_lo) fp8 pair + a ones/zeros column that
  rides the denominator; moving = fp8 exp-probs (et) streamed twice.
- Scores stay fp16 (two heads packed into disjoint 64-row PE groups);
  V/O projections stay fp16 (O-path errors do not attenuate).
Softmax is max-free in the transposed [t, s] domain; the causal mask is a
-30000 pattern accumulated by an identity matmul (exp -> exact 0 in fp8).
Score PSUM is one [128, 4, 512] tile (2 t-tiles x 2 heads) so each exp is a
single 2048-column activation (ACT is the roofline-critical engine).
"""

import numpy as np

B, S, E, H, D = 2, 2048, 1024, 16, 64
HPC = 4              # heads per core
L = HPC * D          # 256 local dims per core
SB = 512             # s-block (softmax/matmul free-dim block)
NSB = S // SB        # 4
TT = 128             # t-tile
NTT = S // TT        # 16
NET = E // 128       # 8 contraction tiles over E

_cache = {}


def _patch_tile_drain():
    """This container's walrus allows only one sync-wait per instruction.
    Split the TileContext tail-drain waits across standalone SP nops."""
    import bass_rust
    import concourse.tile as tile
    from concourse.vector_clock import ScopedClock

    if getattr(tile.TileContext, "_drain_patched", False):
        return

    def _drain_and_barrier(self, tick_clock, wait_clock):
        drain_inst = self.nc.sync.drain()
        wait_clock.add_sem_waits(
            drain_inst.ins, ScopedClock({None: tick_clock.global_clock})
        )
        si = drain_inst.ins.sync_info
        if si is not None and len(si.on_wait) > 1:
            waits = list(si.on_wait)
            drain_inst.ins.sync_info = bass_rust.SyncInfo(
                on_wait=[waits[0]], on_update=list(si.on_update)
            )
            for w in waits[1:]:
                nop = self.nc.sync.nop(nofuse=True)
                nop.ins.sync_info = bass_rust.SyncInfo(on_wait=[w], on_update=[])
        self.nc.all_engine_barrier()
        assert self.sems is not None
        popped = self.nc._tile_sem_poison_stack.pop()
        assert popped is self._sem_poison
        self.nc.clear_and_free_semaphores(list(self.sems.allocated().values()))
        self.nc.all_engine_barrier()

    tile.TileContext._drain_and_barrier = _drain_and_barrier
    tile.TileContext._drain_patched = True


def _split_multi_waits(nc):
    """Move extra per-instruction semaphore waits onto standalone same-engine
    NoOps inserted immediately before the instruction (walrus 1-wait limit)."""
    import bass_rust

    def make_wait_nop(engine_ty, wait):
        eng = None
        for e in (nc.tensor, nc.scalar, nc.vector, nc.gpsimd, nc.sync):
            if e.engine == engine_ty:
                eng = e
                break
        assert eng is not None, f"no engine object for {engine_ty}"
        bi = eng.nop(nofuse=True)
        inst = bi.ins
        bb = nc.cur_bb.bb if nc.cur_bb is not None else None
        if bb is not None and bb.instructions and bb.instructions[-1] is inst:
            bb.instructions.pop()
        inst.sync_info = bass_rust.SyncInfo(on_wait=[wait], on_update=[])
        return inst

    f = nc.m.functions[0]
    for blk in f.blocks:
        new_list = []
        changed = False
        for inst in blk.instructions:
            si = inst.sync_info
            if si is not None and len(si.on_wait) > 1:
                waits = list(si.on_wait)
                for w in waits[:-1]:
                    new_list.append(make_wait_nop(inst.engine, w))
                inst.sync_info = bass_rust.SyncInfo(
                    on_wait=[waits[-1]], on_update=list(si.on_update)
                )
                changed = True
            new_list.append(inst)
        if changed:
            blk.instructions = new_list


def _build(repeat=1):
    import concourse.bass as bass
    import concourse.tile as tile
    from concourse import mybir

    _patch_tile_drain()

    f32 = mybir.dt.float32
    f16 = mybir.dt.float16
    f8 = mybir.dt.float8e4
    EXP = mybir.ActivationFunctionType.Exp
    MULT = mybir.AluOpType.mult
    SUB = mybir.AluOpType.subtract
    DR = mybir.MatmulPerfMode.DoubleRow
    ESCALE = 0.125 / 64.0  # 1/sqrt(D) with the 8x weight scale folded out

    nc = bass.Bass()
    q8 = nc.declare_dram_parameter("q8", [E, S], f8, isOutput=False)
    k8 = nc.declare_dram_parameter("k8", [E, S], f8, isOutput=False)
    vT = nc.declare_dram_parameter("vT", [E, S], f16, isOutput=False)
    wqhl = nc.declare_dram_parameter("wqhl", [E, 2, L], f8, isOutput=False)
    wkhl = nc.declare_dram_parameter("wkhl", [E, 2, L], f8, isOutput=False)
    wv = nc.declare_dram_parameter("wv", [E, L], f16, isOutput=False)
    wo = nc.declare_dram_parameter("wo", [L, E], f16, isOutput=False)
    ident = nc.declare_dram_parameter("ident", [128, 128], f16, isOutput=False)
    bn128 = nc.declare_dram_parameter("bn128", [2, 128, 128], f16, isOutput=False)
    out = nc.declare_dram_parameter("out", [S, E], f16, isOutput=True)

    with tile.TileContext(nc) as tc:
        with (
            tc.tile_pool(name="const", bufs=1) as const,
            tc.tile_pool(name="resid", bufs=1) as resid,
            tc.tile_pool(name="x_in", bufs=1) as x_in,
            tc.tile_pool(name="v_in", bufs=3) as v_in,
            tc.tile_pool(name="expp", bufs=4) as expp,
            tc.tile_pool(name="sm", bufs=3) as smp,
            tc.tile_pool(name="outp", bufs=3) as outp,
        ):
            # ---- resident weights / masks ----
            wq_sb = const.tile([128, NET, 2, L], f8, tag="wq")
            wk_sb = const.tile([128, NET, 2, L], f8, tag="wk")
            wv_sb = const.tile([128, NET, L], f16, tag="wv")
            wo_sb = const.tile([128, 2, E], f16, tag="wo")
            id_sb = const.tile([128, 128], f16, tag="ident")
            bn_sb = const.tile([128, 2, 128], f16, tag="bn128")
            ones_f32 = const.tile([128, 1], f32, tag="ones32")
            ones_r = const.tile([1, 64], f16, tag="onesr")
            nc.gpsimd.dma_start(
                wq_sb[:], wqhl.rearrange("(n p) two l -> p n two l", p=128)
            )
            nc.gpsimd.dma_start(
                wk_sb[:], wkhl.rearrange("(n p) two l -> p n two l", p=128)
            )
            nc.gpsimd.dma_start(wv_sb[:], wv.rearrange("(n p) l -> p n l", p=128))
            nc.gpsimd.dma_start(id_sb[:], ident[:])
            nc.gpsimd.dma_start(bn_sb[:], bn128.rearrange("k p s -> p k s"))
            nc.gpsimd.dma_start(wo_sb[:], wo.rearrange("(h p) e -> p h e", p=128))
            nc.any.memset(ones_f32[:], 1.0)
            nc.vector.tensor_copy(
                ones_r[:], ones_f32[0:1, :].broadcast_to([1, 64])
            )

            # ---- residents ----
            QT = resid.tile([128, 2, S], f16, tag="QT")   # [dim%128, hp, s], 8x
            KT = resid.tile([128, 2, S], f16, tag="KT")
            # V hi/lo fp8 + denominator column: slot 0 = hi (+ones col),
            # slot 1 = lo residual (+zeros col); cols 65:68 pad for 16B steps
            Vn = resid.tile([128, NTT, 2, HPC, 68], f8, tag="Vn")
            PT = resid.tile([128, 2, S], f16, tag="PT")   # normalized attn out.T
            nc.any.memset(Vn[:], 0.0)
            nc.vector.tensor_copy(
                Vn[:, :, 0, :, 64:65],
                ones_f32[:, None, None, :].broadcast_to([128, NTT, HPC, 1]),
            )

            for _rep in range(repeat):
                with (
                    tc.tile_pool(name="ps_sc", bufs=1, space="PSUM") as ps_sc,
                    tc.tile_pool(name="ps_av", bufs=1, space="PSUM") as ps_av,
                    tc.tile_pool(name="ps_gen", bufs=2, space="PSUM") as ps_gen,
                ):
                    xq8 = x_in.tile([128, NET, S], f8, tag="xq")
                    xk8 = x_in.tile([128, NET, S], f8, tag="xk")

                    def load_x(x_sb, x_dram):
                        for half in range(2):
                            ks = slice(half * 4, (half + 1) * 4)
                            nc.sync.dma_start(
                                x_sb[:, ks, :],
                                x_dram.rearrange("(n p) s -> p n s", p=128)[:, ks, :],
                            )

                    def qk_proj_pair(sb_pair):
                        # stationary (w hi/lo) reused across the sb pair so
                        # the 256-col DoubleRow LDWEIGHTS hides under 2 MMs
                        for x8, w_sb, dst in (
                            (xq8, wq_sb, QT),
                            (xk8, wk_sb, KT),
                        ):
                            for hp in range(2):
                                pss = [
                                    (
                                        sb,
                                        ps_gen.tile(
                                            [128, SB], f32, tag="pp", name="pp"
                                        ),
                                    )
                                    for sb in sb_pair
                                ]
                                for kt in range(NET):
                                    for sb, ps in pss:
                                        nc.tensor.matmul(
                                            ps[:],
                                            w_sb[:, kt, :, hp * 128 : (hp + 1) * 128],
                                            x8[
                                                :, kt, None, sb * SB : (sb + 1) * SB
                                            ].broadcast_to([128, 2, SB]),
                                            start=(kt == 0),
                                            stop=(kt == NET - 1),
                                            perf_mode=DR,
                                        )
                                for sb, ps in pss:
                                    nc.vector.tensor_copy(
                                        dst[:, hp, sb * SB : (sb + 1) * SB], ps[:]
                                    )

                    def v_proj(tt):
                        vt = v_in.tile([128, NET, TT], f16, tag="vt")
                        nc.sync.dma_start(
                            vt[:],
                            vT.rearrange("(n p) s -> p n s", p=128)[
                                :, :, tt * TT : (tt + 1) * TT
                            ],
                        )
                        psf = ps_gen.tile([128, SB], f32, tag="pp")
                        ps = psf[:, 0:L]
                        for kk in range(NET):
                            nc.tensor.matmul(
                                ps[:],
                                vt[:, kk, :],
                                wv_sb[:, kk, :],
                                start=(kk == 0),
                                stop=(kk == NET - 1),
                            )
                        src = ps[:].rearrange("p (h d) -> p h d", d=64)
                        nc.vector.tensor_copy(Vn[:, tt, 0, :, 0:64], src)
                        nc.vector.tensor_tensor(
                            Vn[:, tt, 1, :, 0:64], src, Vn[:, tt, 0, :, 0:64], op=SUB
                        )

                    def out_proj(st_list):
                        for st in st_list:
                            for eb in range(E // SB):
                                pso = ps_gen.tile([128, SB], f32, tag="pp")
                                for hp in range(2):
                                    nc.tensor.matmul(
                                        pso[:],
                                        PT[:, hp, st * 128 : (st + 1) * 128],
                                        wo_sb[:, hp, eb * SB : (eb + 1) * SB],
                                        start=(hp == 0),
                                        stop=(hp == 1),
                                    )
                                ot = outp.tile([128, SB], f16, tag="ot")
                                nc.vector.tensor_copy(ot[:], pso[:])
                                nc.sync.dma_start(
                                    out[
                                        st * 128 : (st + 1) * 128,
                                        eb * SB : (eb + 1) * SB,
                                    ],
                                    ot[:],
                                )

                    load_x(xq8, q8)
                    load_x(xk8, k8)
                    for sb in range(NSB):
                        sbs = slice(sb * SB, (sb + 1) * SB)
                        if sb == 0:
                            qk_proj_pair((0, 1))
                        for tt in range(4 * sb, 4 * (sb + 1)):
                            v_proj(tt)
                        if sb == 1:
                            qk_proj_pair((2, 3))

                        # ---- attention for this s-block ----
                        n_tt = (sb + 1) * (SB // TT)  # causal t-tiles
                        for hp in range(2):
                            av0 = ps_av.tile([128, SB], f32, tag="av0")
                            av1 = ps_av.tile([128, SB], f32, tag="av1")
                            for g in range(n_tt // 2):  # pairs of t-tiles
                                # [t, 2 t-tiles x 2 heads, s] score tile:
                                # slots 0,1 = head-even j=0,1; 2,3 = head-odd
                                sc = ps_sc.tile([128, 4, SB], f32, tag="sc")
                                for j in range(2):
                                    tt = 2 * g + j
                                    tts = slice(tt * TT, (tt + 1) * TT)
                                    if tt >= sb * 4:
                                        # diagonal: scores start the group on
                                        # the valid cols; a [128,128] -30000
                                        # above-diag pattern accumulates onto
                                        # the triangle block. Cols [0:no) are
                                        # left unwritten -> exp garbage there
                                        # is never read (AV streams [no:SB)).
                                        kk = tt - sb * 4
                                        no = kk * TT
                                        for h01 in range(2):
                                            nc.tensor.matmul(
                                                sc[:, 2 * h01 + j, no:SB],
                                                KT[
                                                    64 * h01 : 64 * h01 + 64,
                                                    hp, tts,
                                                ],
                                                QT[
                                                    64 * h01 : 64 * h01 + 64,
                                                    hp,
                                                    sb * SB + no : (sb + 1) * SB,
                                                ],
                                                start=True, stop=False,
                                            )
                                            if j == 1:
                                                # fill the 128-col hole the
                                                # pair-wide exp reads below
                                                # this tile's valid range
                                                nc.tensor.matmul(
                                                    sc[
                                                        :, 2 * h01 + j,
                                                        no - TT : no,
                                                    ],
                                                    id_sb[:], bn_sb[:, 0, :],
                                                    start=False, stop=False,
                                                )
                                            nc.tensor.matmul(
                                                sc[:, 2 * h01 + j, no : no + TT],
                                                id_sb[:], bn_sb[:, 1, :],
                                                start=False, stop=True,
                                            )
                                    else:
                                        # heads pack into disjoint PE rows
                                        for h01 in range(2):
                                            nc.tensor.matmul(
                                                sc[:, 2 * h01 + j, :],
                                                KT[
                                                    64 * h01 : 64 * h01 + 64,
                                                    hp, tts,
                                                ],
                                                QT[64 * h01 : 64 * h01 + 64, hp, sbs],
                                                start=True, stop=True,
                                            )
                                et = expp.tile([128, 4, SB], f8, tag="et")
                                # restrict exp to the pair's valid col union
                                npr = max(0, (2 * g - sb * 4) * TT)
                                nc.scalar.activation(
                                    et[:, :, npr:SB], sc[:, :, npr:SB],
                                    EXP, scale=ESCALE,
                                )
                                for h01 in range(2):
                                    av = av0 if h01 == 0 else av1
                                    for j in range(2):
                                        tt = 2 * g + j
                                        st_f = g == 0 and j == 0
                                        sp_f = g == n_tt // 2 - 1 and j == 1
                                        no = max(0, (tt - sb * 4) * TT)
                                        nc.tensor.matmul(
                                            av[0:65, no:SB],
                                            Vn[:, tt, :, 2 * hp + h01, 0:65],
                                            et[
                                                :, 2 * h01 + j, None, no:SB
                                            ].broadcast_to([128, 2, SB - no]),
                                            start=st_f, stop=sp_f,
                                            perf_mode=DR,
                                        )
                            # normalize: PT[po:po+64, hp, sbs] = av[0:64]/av[64]
                            for po, av in ((0, av0), (64, av1)):
                                rdh = smp.tile([1, SB], f16, tag="rdh")
                                with nc.allow_low_precision(
                                    reason="1/den fits fp16; error averages out"
                                ):
                                    nc.vector.reciprocal(rdh[:], av[64:65, :])
                                # broadcast 1/den across 64 partitions via a
                                # K=1 fp16 matmul into the upper half
                                nc.tensor.matmul(
                                    av[64:128, :], ones_r[:], rdh[:],
                                    start=True, stop=True,
                                )
                                bcs = smp.tile([64, SB], f32, tag="bcs")
                                nc.vector.tensor_copy(bcs[:], av[64:128, :])
                                nc.vector.tensor_tensor(
                                    PT[po : po + 64, hp, sbs],
                                    av[0:64, :], bcs[:], op=MULT,
                                )

                        # ---- output projection (one s-block delayed so
                        # it stays available as PE filler; last block inline)
                        if sb > 0:
                            out_proj(
                                list(range(4 * (sb - 1), 4 * sb))
                                + (list(range(12, 16)) if sb == 3 else [])
                            )

    _split_multi_waits(nc)
    return nc


def _get_nc():
    if "nc" not in _cache:
        _cache["nc"] = _build()
    return _cache["nc"]


def _make_runner(nc, n_cores=8):
    """Build a cached jitted SPMD executor (jit once; warm calls are cheap)."""
    import jax
    from jax.sharding import Mesh, PartitionSpec
    from jax.experimental.shard_map import shard_map

    from concourse import mybir
    from concourse.bass2jax import (
        _bass_exec_p,
        install_neuronx_cc_hook,
        partition_id_tensor,
    )

    install_neuronx_cc_hook()
    partition_name = nc.partition_id_tensor.name if nc.partition_id_tensor else None
    in_names, out_names, out_avals, zero_outs = [], [], [], []
    for alloc in nc.m.functions[0].allocations:
        if not isinstance(alloc, mybir.MemoryLocationSet):
            continue
        name = alloc.memorylocations[0].name
        if alloc.kind == "ExternalInput":
            if name != partition_name:
                in_names.append(name)
        elif alloc.kind == "ExternalOutput":
            shape = tuple(alloc.tensor_shape)
            dtype = mybir.dt.np(alloc.dtype)
            out_names.append(name)
            out_avals.append(jax.core.ShapedArray(shape, dtype))
            zero_outs.append(np.zeros(shape, dtype))
    n_params = len(in_names)
    all_in_names = list(in_names) + list(out_names)
    if partition_name is not None:
        all_in_names.append(partition_name)

    def _body(*args):
        operands = list(args)
        if partition_name is not None:
            operands.append(partition_id_tensor())
        return tuple(
            _bass_exec_p.bind(
                *operands,
                out_avals=tuple(out_avals),
                in_names=tuple(all_in_names),
                out_names=tuple(out_names),
                lowering_input_output_aliases=(),
                sim_require_finite=True,
                sim_require_nnan=True,
                nc=nc,
            )
        )

    devices = jax.devices()[:n_cores]
    mesh = Mesh(np.asarray(devices), ("core",))
    in_specs = (PartitionSpec("core"),) * (n_params + len(out_names))
    out_specs = (PartitionSpec("core"),) * len(out_names)
    fn = jax.jit(
        shard_map(
            _body, mesh=mesh, in_specs=in_specs, out_specs=out_specs, check_rep=False
        ),
        keep_unused=True,
    )

    def run(in_maps):
        arrs = [
            np.concatenate([np.asarray(m[name]) for m in in_maps], axis=0)
            for name in in_names
        ]
        zeros = [
            np.zeros((n_cores * z.shape[0], *z.shape[1:]), z.dtype)
            for z in zero_outs
        ]
        outs = fn(*arrs, *zeros)
        per_core = []
        for c in range(n_cores):
            d = {}
            for i, name in enumerate(out_names):
                full = np.asarray(outs[i])
                d[name] = full.reshape(n_cores, full.shape[0] // n_cores, *full.shape[1:])[c]
            per_core.append(d)
        return per_core

    return run


def _get_runner():
    if "run" not in _cache:
        _cache["run"] = _make_runner(_get_nc())
    return _cache["run"]


def _host_inputs(q, k, v, Wq, Wk, Wv, Wo):
    import ml_dtypes

    F8 = ml_dtypes.float8_e4m3
    q = np.asarray(q, dtype=np.float32)
    k = np.asarray(k, dtype=np.float32)
    v = np.asarray(v, dtype=np.float32)
    WoT = np.asarray(Wo, dtype=np.float32).T

    q8b = [q[b].T.astype(F8) for b in range(B)]
    k8b = [k[b].T.astype(F8) for b in range(B)]
    vTb = [v[b].T.astype(np.float16) for b in range(B)]

    def hilo(w):  # [E, L] fp32 -> [E, 2, L] fp8 (hi, residual)
        hi = w.astype(F8)
        lo = (w - hi.astype(np.float32)).astype(F8)
        return np.stack([hi, lo], axis=1)

    wqhl = [hilo(8.0 * np.asarray(Wq, np.float32)[g * L : (g + 1) * L, :].T)
            for g in range(4)]
    wkhl = [hilo(8.0 * np.asarray(Wk, np.float32)[g * L : (g + 1) * L, :].T)
            for g in range(4)]
    wvT = [np.asarray(Wv, np.float32)[g * L : (g + 1) * L, :].T.astype(np.float16)
           for g in range(4)]
    woT = [WoT[g * L : (g + 1) * L, :].astype(np.float16) for g in range(4)]

    ti = np.arange(128)[:, None]
    sj = np.arange(128)[None, :]
    ident = np.eye(128, dtype=np.float16)
    bn128 = np.stack(
        [
            np.full((128, 128), -30000.0, dtype=np.float16),
            np.where(ti > sj, np.float16(-30000.0), np.float16(0.0)),
        ]
    )

    in_maps = []
    for c in range(8):
        b, g = c // 4, c % 4
        in_maps.append(
            {
                "q8": q8b[b], "k8": k8b[b], "vT": vTb[b],
                "wqhl": wqhl[g], "wkhl": wkhl[g], "wv": wvT[g],
                "wo": woT[g], "ident": ident, "bn128": bn128,
            }
        )
    return in_maps


def kernel(q, k, v, Wq, Wk, Wv, Wo, bo):
    run = _get_runner()
    in_maps = _host_inputs(q, k, v, Wq, Wk, Wv, Wo)
    res = run(in_maps)
    out = np.empty((B, S, E), dtype=np.float32)
    bo = np.asarray(bo, dtype=np.float32)
    for b in range(B):
        acc = res[4 * b]["out"].astype(np.float32)
        for g in range(1, 4):
            acc = acc + res[4 * b + g]["out"].astype(np.float32)
        out[b] = acc + bo[None, :]
    return out
